# revision 1
# baseline (speedup 1.0000x reference)
"""Trainium2 Bass kernel for nn_CaptioningRNN (attention-LSTM over T=128 steps).

Sharding: tensor-parallel over the 4H gate dimension across 8 NeuronCores.
Core j owns H-slice j (128 h-rows) of each of the 4 gates (gate columns
{g*1024 + j*128 + i}), so the per-step LSTM cell state (c, h) for that
slice lives entirely on core j. Per step:
  - replicated softmax over the 16 attention scores,
  - local attention slice (DVE mul+reduce against the A slice),
  - AllGather of the bf16 attn^T slices,
  - 16-ktile PE matmul [h; attn] @ [Wh; Wattn] gate-slice (bf16, f32 psum),
  - gates + c/h update (ACT/DVE, f32),
  - PE transpose of h, scores-cross on PE + masked diagonal reduce,
  - AllGather of (hT slice | score partials) in bf16.
The x_t @ Wx gate contribution is computed inline each step as the first
4 ktiles of the same PSUM accumulation (keeps the PE busy during the
AllGather windows); b rides in via a replicated-bias DVE add. All
activations are expressed through Tanh (sigmoid = (1+tanh(x/2))/2,
softmax exp via the tanh identity) so ScalarE keeps one activation
table loaded. Host side does layout-only prep (slicing/transposes/
casts) and the final output assembly out[:, :, Hj] <- core j.
"""
import numpy as np
import ml_dtypes

import bass_rust
import concourse.bass as bass
import concourse.mybir as mybir
from concourse import tile
from concourse.alu_op_type import AluOpType
from concourse.bass_utils import run_bass_kernel_spmd

BF16 = ml_dtypes.bfloat16
F32 = mybir.dt.float32
BF = mybir.dt.bfloat16
AF = mybir.ActivationFunctionType
AX = mybir.AxisListType

N, T, D, H, L, R = 64, 128, 512, 1024, 16, 8
HS, GS = H // R, 4 * H // R  # 128, 512
SCALE = 1.0 / np.sqrt(H)


def _split_waits(nc, cap=1):
    """Walrus here rejects >cap sync waits per instruction; hoist extras
    onto preceding same-engine NOPs."""
    ctr = 0
    for fn in nc.m.functions:
        for bb in fn.blocks:
            out, changed = [], False
            for ins in bb.instructions:
                si = ins.sync_info
                if si is not None and si.on_wait and len(si.on_wait) > cap:
                    waits = list(si.on_wait)
                    extra, keep = waits[:-cap], waits[-cap:]
                    for i in range(0, len(extra), cap):
                        out.append(bass_rust.InstNoOp(
                            name=f"zz_waitsplit_{ctr}", engine=ins.engine,
                            sync_info=bass_rust.SyncInfo(
                                on_wait=extra[i:i + cap], on_update=[])))
                        ctr += 1
                    ins.sync_info = bass_rust.SyncInfo(
                        on_wait=keep, on_update=list(si.on_update or []))
                    changed = True
                out.append(ins)
            if changed:
                bb.instructions = out
    return ctr


def _prep_inputs(x, A, Wx, Wh, Wattn, b):
    x = np.asarray(x, np.float32)
    A_flat = np.asarray(A, np.float32).reshape(N, H, L)
    Wx = np.asarray(Wx, np.float32)
    Wh = np.asarray(Wh, np.float32)
    Wattn = np.asarray(Wattn, np.float32)
    b = np.asarray(b, np.float32)

    W_comb = np.concatenate([Wh, Wattn], axis=0)
    h0 = A_flat.mean(axis=2).astype(np.float32)
    scores0 = (np.einsum('nh,nhl->nl', h0, A_flat) * SCALE).astype(np.float32)
    xT = np.ascontiguousarray(
        x.transpose(1, 2, 0).reshape(T, 4, 128, N)).astype(BF16)
    h0T = np.ascontiguousarray(
        h0.T.reshape(8, 128, N).transpose(1, 0, 2)).astype(BF16)
    eyes = (np.eye(N, dtype=np.float32) * SCALE)

    in_maps = []
    for j in range(R):
        cols = np.array([g * H + j * HS + i for g in range(4) for i in range(HS)])
        hsl = slice(j * HS, (j + 1) * HS)
        in_maps.append({
            "xT": xT,
            "whaj": np.ascontiguousarray(
                W_comb[:, cols].reshape(16, 128, GS)).astype(BF16),
            "wxj": np.ascontiguousarray(
                Wx[:, cols].reshape(4, 128, GS)).astype(BF16),
            "brep": np.tile(b[cols], (128, 1)).astype(np.float32),
            "asT": np.ascontiguousarray(
                A_flat[:, hsl, :].transpose(1, 2, 0)).astype(BF16),
            "anm": np.ascontiguousarray(A_flat[:, hsl, :]).astype(BF16),
            "eyes": eyes.astype(BF16),
            "iden": np.eye(128, dtype=np.float32),
            "h0T": h0T,
            "c0": np.ascontiguousarray(h0[:, hsl]),
            "s0": scores0,
        })
    return in_maps


def _build():
    nc = bass.Bass("TRN2", target_bir_lowering=False, debug=False, num_devices=R)
    rg = [list(range(R))]

    xT_d = nc.dram_tensor("xT", [T, 4, 128, N], BF, kind="ExternalInput")
    whaj_d = nc.dram_tensor("whaj", [16, 128, GS], BF, kind="ExternalInput")
    wxj_d = nc.dram_tensor("wxj", [4, 128, GS], BF, kind="ExternalInput")
    brep_d = nc.dram_tensor("brep", [128, GS], F32, kind="ExternalInput")
    asT_d = nc.dram_tensor("asT", [128, L, N], BF, kind="ExternalInput")
    anm_d = nc.dram_tensor("anm", [N, HS, L], BF, kind="ExternalInput")
    eyes_d = nc.dram_tensor("eyes", [N, N], BF, kind="ExternalInput")
    iden_d = nc.dram_tensor("iden", [128, 128], F32, kind="ExternalInput")
    h0T_d = nc.dram_tensor("h0T", [128, 8, N], BF, kind="ExternalInput")
    c0_d = nc.dram_tensor("c0", [N, HS], F32, kind="ExternalInput")
    s0_d = nc.dram_tensor("s0", [N, L], F32, kind="ExternalInput")
    out_d = nc.dram_tensor("out", [N, T, HS], F32, kind="ExternalOutput")

    with tile.TileContext(nc) as tc:
        with tc.tile_pool(name="const", bufs=1) as cp, \
             tc.tile_pool(name="state", bufs=1) as st, \
             tc.tile_pool(name="dram", bufs=2, space="DRAM") as dp:

            whaj = cp.tile([128, 16, GS], BF, name="whaj")
            wxj = cp.tile([128, 4, GS], BF, name="wxj")
            brep = cp.tile([128, GS], F32, name="brep")
            nc.sync.dma_start(out=wxj[:, :, :], in_=wxj_d.rearrange("k p g -> p k g"))
            nc.sync.dma_start(out=brep[:, :], in_=brep_d[:, :])
            asT = cp.tile([128, L, N], BF, name="asT")
            anm = cp.tile([N, HS, L], BF, name="anm")
            eyes = cp.tile([N, N], BF, name="eyes")
            iden = cp.tile([128, 128], F32, name="iden")
            nc.sync.dma_start(out=whaj[:, :, :], in_=whaj_d.rearrange("k p g -> p k g"))
            nc.sync.dma_start(out=asT[:, :, :], in_=asT_d[:, :, :])
            nc.sync.dma_start(out=anm[:, :, :], in_=anm_d[:, :, :])
            nc.sync.dma_start(out=eyes[:, :], in_=eyes_d[:, :])
            nc.sync.dma_start(out=iden[:, :], in_=iden_d[:, :])

            c = st.tile([N, HS], F32, name="c")
            nc.sync.dma_start(out=c[:, :], in_=c0_d[:, :])

            with tc.tile_pool(name="wk", bufs=2) as wk, \
                 tc.tile_pool(name="ps_a", bufs=2, space="PSUM") as ps_a, \
                 tc.tile_pool(name="ps_x", bufs=1, space="PSUM") as ps_x, \
                 tc.tile_pool(name="ps_t", bufs=1, space="PSUM") as ps_t:

                hT_full = wk.tile([128, 8, N], BF, name="hT0", tag="hT_full")
                nc.sync.dma_start(out=hT_full[:, :, :], in_=h0T_d[:, :, :])
                scores = wk.tile([N, L], F32, name="scores0", tag="scores")
                nc.sync.dma_start(out=scores[:, :], in_=s0_d[:, :])

                for t in range(T):
                    # softmax without max-sub (scores bounded); exp/sigmoid via
                    # tanh so ScalarE keeps ONE activation table loaded:
                    # exp(x) = (1 + tanh(x/2)) / (1 - tanh(x/2))
                    th = wk.tile([N, L], F32, name="th", tag="th")
                    nc.scalar.activation(th[:, :], scores[:, :], AF.Tanh, scale=0.5)
                    den = wk.tile([N, L], F32, name="den", tag="den")
                    nc.vector.tensor_scalar(out=den[:, :], in0=th[:, :],
                                            scalar1=-1.0, scalar2=1.0,
                                            op0=AluOpType.mult, op1=AluOpType.add)
                    rden = wk.tile([N, L], F32, name="rden", tag="rden")
                    nc.vector.reciprocal(out=rden[:, :], in_=den[:, :])
                    num = wk.tile([N, L], F32, name="num", tag="num")
                    nc.vector.tensor_scalar(out=num[:, :], in0=th[:, :],
                                            scalar1=1.0, scalar2=None,
                                            op0=AluOpType.add)
                    e = wk.tile([N, L], F32, name="e", tag="e")
                    nc.vector.tensor_mul(out=e[:, :], in0=num[:, :], in1=rden[:, :])
                    se = wk.tile([N, 1], F32, name="se", tag="se")
                    nc.vector.reduce_sum(out=se[:, :], in_=e[:, :], axis=AX.X)
                    rse = wk.tile([N, 1], F32, name="rse", tag="rse")
                    nc.vector.reciprocal(out=rse[:, :], in_=se[:, :])
                    w = wk.tile([N, L], F32, name="w", tag="w")
                    nc.vector.tensor_scalar(out=w[:, :], in0=e[:, :],
                                            scalar1=rse[:, 0:1], scalar2=None,
                                            op0=AluOpType.mult)
                    w_bf = wk.tile([N, L], BF, name="w_bf", tag="w_bf")
                    nc.vector.tensor_copy(out=w_bf[:, :], in_=w[:, :])
                    atm = wk.tile([N, HS, L], BF, name="atm", tag="atm")
                    nc.vector.tensor_tensor(
                        out=atm[:, :, :], in0=anm[:, :, :],
                        in1=w_bf[:, None, :].broadcast_to((N, HS, L)),
                        op=AluOpType.mult)
                    attn_j = wk.tile([N, HS], F32, name="attn_j", tag="attn_j")
                    nc.vector.reduce_sum(out=attn_j[:, :], in_=atm[:, :, :],
                                         axis=AX.X)
                    pt1 = ps_t.tile([128, N], F32, name="pt1", tag="pt1")
                    nc.tensor.transpose(pt1[:, :], attn_j[:, :], iden[0:N, 0:N])
                    attnT_bf = wk.tile([128, N], BF, name="attnT_bf", tag="attnT_bf")
                    nc.vector.tensor_copy(out=attnT_bf[:, :], in_=pt1[:, :])
                    sendB = dp.tile([128, N], BF, name="sendB", tag="sendB")
                    recvB = dp.tile([R, 128, N], BF, name="recvB", tag="recvB",
                                    addr_space="Shared")
                    nc.sync.dma_start(out=sendB[:, :], in_=attnT_bf[:, :])
                    nc.gpsimd.collective_compute(
                        "AllGather", AluOpType.bypass, replica_groups=rg,
                        ins=[sendB[:, :].opt()], outs=[recvB[:, :, :].opt()])
                    attnT_full = wk.tile([128, 8, N], BF, name="attnT_full",
                                         tag="attnT_full")
                    nc.sync.dma_start(out=attnT_full[:, :, :],
                                      in_=recvB.rearrange("r p n -> p r n"))
                    pa = ps_a.tile([N, GS], F32, name="pa", tag="pa")
                    pax = ps_a.tile([N, GS], F32, name="pax", tag="pax")
                    xtile = wk.tile([128, 4, N], BF, name="xtile", tag="xtile")
                    nc.sync.dma_start(out=xtile[:, :, :],
                                      in_=xT_d[t].rearrange("k p n -> p k n"))
                    for kt in range(4):
                        nc.tensor.matmul(pax[:, :], xtile[:, kt, :], wxj[:, kt, :],
                                         start=(kt == 0), stop=(kt == 3))
                    for kt in range(8):
                        nc.tensor.matmul(pa[:, :], hT_full[:, kt, :],
                                         whaj[:, kt, :],
                                         start=(kt == 0), stop=False)
                    for kt in range(8):
                        nc.tensor.matmul(pa[:, :], attnT_full[:, kt, :],
                                         whaj[:, 8 + kt, :],
                                         start=False, stop=(kt == 7))
                    s_g = wk.tile([N, GS], F32, name="s_g", tag="s_g")
                    nc.vector.tensor_add(out=s_g[:, :], in0=pa[:, :],
                                         in1=brep[0:N, :])
                    nc.vector.tensor_add(out=s_g[:, :], in0=pax[:, :],
                                         in1=s_g[:, :])
                    sg3 = wk.tile([N, 3 * HS], F32, name="sg3", tag="sg3")
                    nc.scalar.activation(sg3[:, :], s_g[:, 0:3 * HS], AF.Tanh,
                                         scale=0.5)
                    sig = wk.tile([N, 3 * HS], F32, name="sig", tag="sig")
                    nc.vector.tensor_scalar(out=sig[:, :], in0=sg3[:, :],
                                            scalar1=1.0, scalar2=0.5,
                                            op0=AluOpType.add, op1=AluOpType.mult)
                    gt = wk.tile([N, HS], F32, name="gt", tag="gt")
                    nc.scalar.activation(gt[:, :], s_g[:, 3 * HS:4 * HS], AF.Tanh)
                    t1 = wk.tile([N, HS], F32, name="t1", tag="t1")
                    nc.vector.tensor_mul(out=t1[:, :], in0=sig[:, 0:HS],
                                         in1=gt[:, :])
                    nc.vector.tensor_mul(out=c[:, :], in0=sig[:, HS:2 * HS],
                                         in1=c[:, :])
                    nc.vector.tensor_add(out=c[:, :], in0=c[:, :], in1=t1[:, :])
                    tanc = wk.tile([N, HS], F32, name="tanc", tag="tanc")
                    nc.scalar.activation(tanc[:, :], c[:, :], AF.Tanh)
                    h_j = wk.tile([N, HS], F32, name="h_j", tag="h_j")
                    nc.vector.tensor_mul(out=h_j[:, :], in0=sig[:, 2 * HS:3 * HS],
                                         in1=tanc[:, :])
                    nc.sync.dma_start(out=out_d[:, t, :], in_=h_j[:, :])
                    if t == T - 1:
                        break
                    pt2 = ps_t.tile([128, N], F32, name="pt2", tag="pt2")
                    nc.tensor.transpose(pt2[:, :], h_j[:, :], iden[0:N, 0:N])
                    hT_bf = wk.tile([128, N], BF, name="hT_bf", tag="hT_bf")
                    nc.vector.tensor_copy(out=hT_bf[:, :], in_=pt2[:, :])
                    px = ps_x.tile([N, L, N], F32, name="px", tag="px")
                    nc.tensor.matmul(px[:, 0:8, :], hT_bf[:, :], asT[:, 0:8, :],
                                     start=True, stop=True)
                    nc.tensor.matmul(px[:, 8:16, :], hT_bf[:, :], asT[:, 8:16, :],
                                     start=True, stop=True)
                    pxs = wk.tile([N, L, N], BF, name="pxs", tag="pxs")
                    nc.vector.tensor_copy(out=pxs[:, :, :], in_=px[:, :, :])
                    msk = wk.tile([N, L, N], BF, name="msk", tag="msk")
                    nc.vector.tensor_tensor(
                        out=msk[:, :, :], in0=pxs[:, :, :],
                        in1=eyes[:, None, :].broadcast_to((N, L, N)),
                        op=AluOpType.mult)
                    spart_bf = wk.tile([N, L], BF, name="spart_bf", tag="spart_bf")
                    with nc.allow_low_precision(reason="masked diag pick, single nonzero"):
                        nc.vector.reduce_sum(out=spart_bf[:, :], in_=msk[:, :, :],
                                             axis=AX.X)
                    sendA = dp.tile([9216], BF, name="sendA", tag="sendA")
                    recvA = dp.tile([R, 9216], BF, name="recvA", tag="recvA",
                                    addr_space="Shared")
                    nc.sync.dma_start(
                        out=sendA[0:8192].rearrange("(p n) -> p n", p=128),
                        in_=hT_bf[:, :])
                    nc.sync.dma_start(
                        out=sendA[8192:9216].rearrange("(n l) -> n l", n=N),
                        in_=spart_bf[:, :])
                    nc.gpsimd.collective_compute(
                        "AllGather", AluOpType.bypass, replica_groups=rg,
                        ins=[sendA[:].opt()], outs=[recvA[:, :].opt()])
                    hT_full = wk.tile([128, 8, N], BF, name="hT_full",
                                      tag="hT_full")
                    nc.sync.dma_start(
                        out=hT_full[:, :, :],
                        in_=recvA[:, 0:8192].rearrange("r (p n) -> p r n", p=128))
                    sparts = wk.tile([N, 8, L], BF, name="sparts", tag="sparts")
                    nc.sync.dma_start(
                        out=sparts[:, :, :],
                        in_=recvA[:, 8192:9216].rearrange("r (n l) -> n r l", n=N))
                    scores = wk.tile([N, L], F32, name="scores", tag="scores")
                    nc.vector.reduce_sum(out=scores[:, :],
                                         in_=sparts.rearrange("n r l -> n l r"),
                                         axis=AX.X)

    _split_waits(nc, cap=1)
    return nc


_NC_CACHE = None


def kernel(**inputs) -> np.ndarray:
    global _NC_CACHE
    in_maps = _prep_inputs(**inputs)
    if _NC_CACHE is None:
        _NC_CACHE = _build()
    res = run_bass_kernel_spmd(_NC_CACHE, in_maps, core_ids=list(range(R)))
    out = np.zeros((N, T, H), dtype=np.float32)
    for j, r in enumerate(res.results):
        out[:, :, j * HS:(j + 1) * HS] = np.asarray(r["out"]).reshape(N, T, HS)
    return out



# revision 19
# speedup vs baseline: 1.0857x; 1.0857x over previous
"""Trainium2 Bass kernel for nn_CaptioningRNN (attention-LSTM over T=128 steps).

Sharding: tensor-parallel over the 4H gate dimension across 8 NeuronCores.
Core j owns H-slice j (128 h-rows) of each of the 4 gates, so the per-step
LSTM cell state (c, h) for that slice lives entirely on core j.

Key structure (v2 — ONE collective per step):
  - The attention contribution to the gates is reparametrized through
    B[n, l, g] = sum_h A[n, h, l] * Wattn[h, g]  (precomputed on-device,
    per-core gate slice, packed [128, 256, 16] across two gate halves), so
    per step  attn @ Wattn == sum_l softmax(scores)[n, l] * B[n, l, :] —
    a small DVE contraction over L=16 on all 128 partitions. This removes
    the per-step attn^T AllGather entirely.
  - Per step each core broadcasts (hT slice | score partials) in ONE bf16
    AllGather; scores are summed from partials, softmax is computed
    duplicated on 128 partitions, and the attention-gate term is injected
    into the PSUM gate accumulation via two tiny identity matmuls.
  - The x_t @ Wx gate ktiles + a K=1 bias ktile ride in the same PSUM
    accumulation during the AllGather window.
  - All activations go through Tanh only (sigmoid/exp via tanh identities)
    so ScalarE keeps one activation table loaded.
  - h is accumulated in SBUF (bf16) and written out with a single DMA at
    the end. Host side does layout-only prep and output assembly.
"""
import numpy as np
import ml_dtypes

import bass_rust
import concourse.bass as bass
import concourse.mybir as mybir
from concourse import tile
from concourse.alu_op_type import AluOpType
from concourse.bass_utils import run_bass_kernel_spmd

BF16 = ml_dtypes.bfloat16
F32 = mybir.dt.float32
BF = mybir.dt.bfloat16
AF = mybir.ActivationFunctionType
AX = mybir.AxisListType

N, T, D, H, L, R = 64, 128, 512, 1024, 16, 8
HS, GS = H // R, 4 * H // R  # 128, 512
GH = GS // 2  # 256: gate-half width for the packed B layout
SCALE = 1.0 / np.sqrt(H)


def _split_waits(nc, cap=1):
    """Walrus here rejects >cap sync waits per instruction; hoist extras
    onto preceding same-engine NOPs."""
    ctr = 0
    for fn in nc.m.functions:
        for bb in fn.blocks:
            out, changed = [], False
            for ins in bb.instructions:
                si = ins.sync_info
                if si is not None and si.on_wait and len(si.on_wait) > cap:
                    waits = list(si.on_wait)
                    extra, keep = waits[:-cap], waits[-cap:]
                    for i in range(0, len(extra), cap):
                        out.append(bass_rust.InstNoOp(
                            name=f"zz_waitsplit_{ctr}", engine=ins.engine,
                            sync_info=bass_rust.SyncInfo(
                                on_wait=extra[i:i + cap], on_update=[])))
                        ctr += 1
                    ins.sync_info = bass_rust.SyncInfo(
                        on_wait=keep, on_update=list(si.on_update or []))
                    changed = True
                out.append(ins)
            if changed:
                bb.instructions = out
    return ctr


def _prep_inputs(x, A, Wx, Wh, Wattn, b):
    x = np.asarray(x, np.float32)
    A_flat = np.asarray(A, np.float32).reshape(N, H, L)
    Wx = np.asarray(Wx, np.float32)
    Wh = np.asarray(Wh, np.float32)
    Wattn = np.asarray(Wattn, np.float32)
    b = np.asarray(b, np.float32)

    h0 = A_flat.mean(axis=2).astype(np.float32)
    scores0 = (np.einsum('nh,nhl->nl', h0, A_flat) * SCALE).astype(np.float32)
    xT = np.ascontiguousarray(
        x.transpose(1, 2, 0).reshape(T, 4, 128, N)).astype(BF16)
    h0T = np.ascontiguousarray(
        h0.T.reshape(8, 128, N).transpose(1, 0, 2)).astype(BF16)
    # A arranged for the on-device B precompute: lhsT chunks
    # [hh (partition), k (h-chunk), (m, nn, l)] with n = 8m + nn.
    abT = np.ascontiguousarray(
        A_flat.reshape(8, 8, 8, 128, L)            # [m, nn, k, hh, l]
        .transpose(2, 3, 0, 1, 4)                  # [k, hh, m, nn, l]
        .reshape(8, 128, 8 * 8 * L)).astype(BF16)  # [k, hh, 1024]
    idnstk = np.vstack([np.eye(N, dtype=np.float32)] * 2)  # [128, 64]
    s02 = np.tile(scores0, (2, 1))  # [128, 16]

    in_maps = []
    for j in range(R):
        cols = np.array([g * H + j * HS + i for g in range(4) for i in range(HS)])
        hsl = slice(j * HS, (j + 1) * HS)
        in_maps.append({
            "xT": xT,
            "whj": np.ascontiguousarray(
                Wh[:, cols].reshape(8, 128, GS)).astype(BF16),
            "wxj": np.ascontiguousarray(
                Wx[:, cols].reshape(4, 128, GS)).astype(BF16),
            "wanj": np.ascontiguousarray(
                Wattn[:, cols].reshape(8, 128, GS)).astype(BF16),
            # bias folded into B (exact: softmax weights sum to 1); packed
            # rows: partition p = n + 64*gh holds gate cols [gh*GH, gh*GH+GH)
            "b2rep": np.ascontiguousarray(
                np.tile(b[cols].reshape(2, GH), (N, 1)).reshape(N, 2, GH)
                .transpose(1, 0, 2).reshape(128, GH)).astype(np.float32),
            "abT": abT,
            "alh": np.ascontiguousarray(
                A_flat[:, hsl, :].transpose(0, 2, 1) * SCALE).astype(BF16),
            "idnstk": idnstk.astype(BF16),
            "h0T": h0T,
            "c0": np.ascontiguousarray(h0[:, hsl]),
            "s02": s02,
        })
    return in_maps


def _build():
    nc = bass.Bass("TRN2", target_bir_lowering=False, debug=False, num_devices=R)
    rg = [list(range(R))]

    xT_d = nc.dram_tensor("xT", [T, 4, 128, N], BF, kind="ExternalInput")
    whj_d = nc.dram_tensor("whj", [8, 128, GS], BF, kind="ExternalInput")
    wxj_d = nc.dram_tensor("wxj", [4, 128, GS], BF, kind="ExternalInput")
    wanj_d = nc.dram_tensor("wanj", [8, 128, GS], BF, kind="ExternalInput")
    b2rep_d = nc.dram_tensor("b2rep", [128, GH], F32, kind="ExternalInput")
    abT_d = nc.dram_tensor("abT", [8, 128, 8 * 8 * L], BF, kind="ExternalInput")
    alh_d = nc.dram_tensor("alh", [N, L, HS], BF, kind="ExternalInput")
    idn_d = nc.dram_tensor("idnstk", [128, N], BF, kind="ExternalInput")
    h0T_d = nc.dram_tensor("h0T", [128, 8, N], BF, kind="ExternalInput")
    c0_d = nc.dram_tensor("c0", [N, HS], F32, kind="ExternalInput")
    s02_d = nc.dram_tensor("s02", [128, L], F32, kind="ExternalInput")
    out_d = nc.dram_tensor("out", [N, T, HS], BF, kind="ExternalOutput")

    with tile.TileContext(nc) as tc:
        with tc.tile_pool(name="const", bufs=1) as cp, \
             tc.tile_pool(name="state", bufs=1) as st, \
             tc.tile_pool(name="dram", bufs=2, space="DRAM") as dp:

            whj = cp.tile([128, 8, GS], BF, name="whj")
            wxj = cp.tile([128, 4, GS], BF, name="wxj")
            alh = cp.tile([N, L, HS], BF, name="alh")
            idnstk = cp.tile([128, N], BF, name="idnstk")
            B2 = cp.tile([128, L, GH], BF, name="B2")
            nc.sync.dma_start(out=whj[:, :, :], in_=whj_d.rearrange("k p g -> p k g"))
            nc.sync.dma_start(out=wxj[:, :, :], in_=wxj_d.rearrange("k p g -> p k g"))
            nc.sync.dma_start(out=alh[:, :, :], in_=alh_d[:, :, :])
            nc.sync.dma_start(out=idnstk[:, :], in_=idn_d[:, :])

            c = st.tile([N, HS], F32, name="c")
            nc.sync.dma_start(out=c[:, :], in_=c0_d[:, :])
            houtT = st.tile([N, T, HS], BF, name="houtT")
            # block-diagonal staging for the attn-gate PSUM injection:
            # rows 0:64 carry gate cols 0:GH, rows 64:128 carry GH:GS;
            # off-diagonal blocks stay zero forever.
            ga_bd = st.tile([128, GS], BF, name="ga_bd")
            nc.vector.memset(ga_bd[:, :], 0.0)

            # ---- B precompute: B2[(gh n), l, gp] = B[n, l, gh*GH+gp] ----
            with tc.tile_pool(name="btmp", bufs=1) as bt, \
                 tc.tile_pool(name="ps_b", bufs=2, space="PSUM") as psb:
                b2_d = dp.tile([8, 128, GS], BF, name="b2stage")
                abT = bt.tile([128, 8, 8 * 8 * L], BF, name="abT")
                wan = bt.tile([128, 8, GS], BF, name="wan")
                b2rep = bt.tile([128, GH], F32, name="b2rep")
                nc.sync.dma_start(out=abT[:, :, :],
                                  in_=abT_d.rearrange("k p q -> p k q"))
                nc.sync.dma_start(out=wan[:, :, :],
                                  in_=wanj_d.rearrange("k p g -> p k g"))
                nc.sync.dma_start(out=b2rep[:, :], in_=b2rep_d[:, :])
                for m in range(8):
                    psB = psb.tile([128, GS], F32, name="psB", tag="psB")
                    for k in range(8):
                        nc.tensor.matmul(psB[:, :],
                                         abT[:, k, m * 128:(m + 1) * 128],
                                         wan[:, k, :],
                                         start=(k == 0), stop=(k == 7))
                    chunk = bt.tile([128, GS], BF, name=f"chunk{m}")
                    nc.vector.tensor_copy(out=chunk[:, :], in_=psB[:, :])
                    nc.sync.dma_start(out=b2_d[m], in_=chunk[:, :])
                b2view = b2_d.rearrange("m (nn l) (gh gp) -> gh (m nn) l gp",
                                        nn=8, l=L, gh=2, gp=GH)
                nc.sync.dma_start(out=B2[0:N, :, :], in_=b2view[0])
                nc.sync.dma_start(out=B2[N:128, :, :], in_=b2view[1])
                # fold the gate bias into B (softmax weights sum to 1)
                nc.vector.tensor_tensor(
                    out=B2[:, :, :], in0=B2[:, :, :],
                    in1=b2rep[:, None, :].broadcast_to((128, L, GH)),
                    op=AluOpType.add)

            with tc.tile_pool(name="wk", bufs=2) as wk, \
                 tc.tile_pool(name="ps_a", bufs=2, space="PSUM") as ps_a, \
                 tc.tile_pool(name="ps_t", bufs=2, space="PSUM") as ps_t:

                hT_full = wk.tile([128, 8, N], BF, name="hT0", tag="hT_full")
                nc.sync.dma_start(out=hT_full[:, :, :], in_=h0T_d[:, :, :])
                scores2 = wk.tile([128, L], F32, name="scores0", tag="scores2")
                nc.sync.dma_start(out=scores2[:, :], in_=s02_d[:, :])

                for t in range(T):
                    # ---- gate preacts accumulate in PSUM: x, bias, h ----
                    xtile = wk.tile([128, 4, N], BF, name="xtile", tag="xtile")
                    nc.sync.dma_start(out=xtile[:, :, :],
                                      in_=xT_d[t].rearrange("k p n -> p k n"))
                    pa = ps_a.tile([N, GS], F32, name="pa", tag="pa")
                    for kt in range(4):
                        nc.tensor.matmul(pa[:, :], xtile[:, kt, :], wxj[:, kt, :],
                                         start=(kt == 0), stop=False)
                    for kt in range(8):
                        nc.tensor.matmul(pa[:, :], hT_full[:, kt, :],
                                         whj[:, kt, :], start=False, stop=False)

                    # ---- softmax on duplicated [128, L] scores ----
                    # exp(x) = (1 + tanh(x/2)) / (1 - tanh(x/2))
                    th2 = wk.tile([128, L], F32, name="th2", tag="th2")
                    nc.scalar.activation(th2[:, :], scores2[:, :], AF.Tanh,
                                         scale=0.5)
                    den2 = wk.tile([128, L], F32, name="den2", tag="den2")
                    nc.vector.tensor_scalar(out=den2[:, :], in0=th2[:, :],
                                            scalar1=-1.0, scalar2=1.0,
                                            op0=AluOpType.mult, op1=AluOpType.add)
                    rden2 = wk.tile([128, L], F32, name="rden2", tag="rden2")
                    nc.vector.reciprocal(out=rden2[:, :], in_=den2[:, :])
                    e2 = wk.tile([128, L], BF, name="e2", tag="e2")
                    se2 = wk.tile([128, 1], F32, name="se2", tag="se2")
                    nc.vector.scalar_tensor_tensor(
                        out=e2[:, :], in0=th2[:, :], scalar=1.0,
                        in1=rden2[:, :], op0=AluOpType.add, op1=AluOpType.mult,
                        accum_out=se2[:, :])
                    rse2 = wk.tile([128, 1], F32, name="rse2", tag="rse2")
                    nc.vector.reciprocal(out=rse2[:, :], in_=se2[:, :])

                    # ---- attention-gate term via B: sum_l w_l * B_l ----
                    ga_m = wk.tile([128, L, GH], BF, name="ga_m", tag="ga_m")
                    nc.vector.scalar_tensor_tensor(
                        out=ga_m[:, :, :], in0=B2[:, :, :],
                        scalar=rse2[:, 0:1],
                        in1=e2[:, :, None].broadcast_to((128, L, GH)),
                        op0=AluOpType.mult, op1=AluOpType.mult)
                    with nc.allow_low_precision(reason="16-term attn mix, bf16 ok"):
                        nc.vector.reduce_sum(
                            out=ga_bd[0:N, 0:GH],
                            in_=ga_m.rearrange("p l g -> p g l")[0:N],
                            axis=AX.X)
                        nc.vector.reduce_sum(
                            out=ga_bd[N:128, GH:GS],
                            in_=ga_m.rearrange("p l g -> p g l")[N:128],
                            axis=AX.X)
                    # inject into the PSUM gate accumulation (identity matmul)
                    nc.tensor.matmul(pa[:, :], idnstk[:, :], ga_bd[:, :],
                                     start=False, stop=True)

                    # ---- gates: i,f,o = sigmoid, g = tanh (all via Tanh) ----
                    th3 = wk.tile([N, 3 * HS], F32, name="th3", tag="th3")
                    nc.scalar.activation(th3[:, :], pa[:, 0:3 * HS], AF.Tanh,
                                         scale=0.5)
                    sig = wk.tile([N, 3 * HS], F32, name="sig", tag="sig")
                    nc.vector.tensor_scalar(out=sig[:, :], in0=th3[:, :],
                                            scalar1=1.0, scalar2=0.5,
                                            op0=AluOpType.add, op1=AluOpType.mult)
                    gt = wk.tile([N, HS], F32, name="gt", tag="gt")
                    nc.scalar.activation(gt[:, :], pa[:, 3 * HS:4 * HS], AF.Tanh)
                    t1 = wk.tile([N, HS], F32, name="t1", tag="t1")
                    nc.vector.tensor_mul(out=t1[:, :], in0=sig[:, 0:HS],
                                         in1=gt[:, :])
                    nc.vector.tensor_mul(out=c[:, :], in0=sig[:, HS:2 * HS],
                                         in1=c[:, :])
                    nc.vector.tensor_add(out=c[:, :], in0=c[:, :], in1=t1[:, :])
                    tanc = wk.tile([N, HS], F32, name="tanc", tag="tanc")
                    nc.scalar.activation(tanc[:, :], c[:, :], AF.Tanh)
                    nc.vector.tensor_mul(out=houtT[:, t, :],
                                         in0=sig[:, 2 * HS:3 * HS],
                                         in1=tanc[:, :])
                    if t == T - 1:
                        break

                    # ---- next-step comms: hT slice + score partials ----
                    pt = ps_t.tile([128, N], BF, name="pt", tag="pt")
                    nc.tensor.transpose(pt[:, :], houtT[:, t, :], idnstk[0:N, :])
                    hT_bf = wk.tile([128, N], BF, name="hT_bf", tag="hT_bf")
                    nc.vector.tensor_copy(out=hT_bf[:, :], in_=pt[:, :])
                    atm = wk.tile([N, L, HS], BF, name="atm", tag="atm")
                    nc.vector.tensor_tensor(
                        out=atm[:, :, :], in0=alh[:, :, :],
                        in1=houtT[:, t, None, :].broadcast_to((N, L, HS)),
                        op=AluOpType.mult)
                    spart = wk.tile([N, L], BF, name="spart", tag="spart")
                    with nc.allow_low_precision(reason="score partials, bf16 ok"):
                        nc.vector.reduce_sum(out=spart[:, :], in_=atm[:, :, :],
                                             axis=AX.X)

                    sendA = dp.tile([8192 + N * L], BF, name="sendA", tag="sendA")
                    recvA = dp.tile([R, 8192 + N * L], BF, name="recvA",
                                    tag="recvA", addr_space="Shared")
                    nc.sync.dma_start(
                        out=sendA[0:8192].rearrange("(p n) -> p n", p=128),
                        in_=hT_bf[:, :])
                    nc.sync.dma_start(
                        out=sendA[8192:8192 + N * L].rearrange("(n l) -> n l", n=N),
                        in_=spart[:, :])
                    nc.gpsimd.collective_compute(
                        "AllGather", AluOpType.bypass, replica_groups=rg,
                        ins=[sendA[:].opt()], outs=[recvA[:, :].opt()])
                    hT_full = wk.tile([128, 8, N], BF, name="hT_full",
                                      tag="hT_full")
                    nc.sync.dma_start(
                        out=hT_full[:, :, :],
                        in_=recvA[:, 0:8192].rearrange("r (p n) -> p r n", p=128))
                    sp2 = wk.tile([128, 8, L], BF, name="sp2", tag="sp2")
                    nc.sync.dma_start(
                        out=sp2[0:N, :, :],
                        in_=recvA[:, 8192:8192 + N * L].rearrange(
                            "r (n l) -> n r l", n=N))
                    nc.sync.dma_start(
                        out=sp2[N:128, :, :],
                        in_=recvA[:, 8192:8192 + N * L].rearrange(
                            "r (n l) -> n r l", n=N))
                    scores2 = wk.tile([128, L], F32, name="scores2",
                                      tag="scores2")
                    nc.vector.reduce_sum(out=scores2[:, :],
                                         in_=sp2.rearrange("p r l -> p l r"),
                                         axis=AX.X)

                nc.sync.dma_start(out=out_d[:, :, :], in_=houtT[:, :, :])

    _split_waits(nc, cap=1)
    return nc


_NC_CACHE = None


def kernel(**inputs) -> np.ndarray:
    global _NC_CACHE
    in_maps = _prep_inputs(**inputs)
    if _NC_CACHE is None:
        _NC_CACHE = _build()
    res = run_bass_kernel_spmd(_NC_CACHE, in_maps, core_ids=list(range(R)))
    out = np.zeros((N, T, H), dtype=np.float32)
    for j, r in enumerate(res.results):
        out[:, :, j * HS:(j + 1) * HS] = \
            np.asarray(r["out"]).astype(np.float32).reshape(N, T, HS)
    return out


# revision 22
# speedup vs baseline: 1.3013x; 1.1986x over previous
"""Trainium2 Bass kernel for nn_CaptioningRNN (attention-LSTM over T=128 steps).

Sharding: tensor-parallel over the 4H gate dimension across 8 NeuronCores.
Core j owns H-slice j (128 h-rows) of each of the 4 gates, so the per-step
LSTM cell state (c, h) for that slice lives entirely on core j.

Key structure (v2 — ONE collective per step):
  - The attention contribution to the gates is reparametrized through
    B[n, l, g] = sum_h A[n, h, l] * Wattn[h, g]  (precomputed on-device,
    per-core gate slice, packed [128, 256, 16] across two gate halves), so
    per step  attn @ Wattn == sum_l softmax(scores)[n, l] * B[n, l, :] —
    a small DVE contraction over L=16 on all 128 partitions. This removes
    the per-step attn^T AllGather entirely.
  - Per step each core broadcasts (hT slice | score partials) in ONE bf16
    AllGather; scores are summed from partials, softmax is computed
    duplicated on 128 partitions, and the attention-gate term is injected
    into the PSUM gate accumulation via two tiny identity matmuls.
  - The x_t @ Wx gate ktiles + a K=1 bias ktile ride in the same PSUM
    accumulation during the AllGather window.
  - All activations go through Tanh only (sigmoid/exp via tanh identities)
    so ScalarE keeps one activation table loaded.
  - h is accumulated in SBUF (bf16) and written out with a single DMA at
    the end. Host side does layout-only prep and output assembly.
"""
import numpy as np
import ml_dtypes

import bass_rust
import concourse.bass as bass
import concourse.mybir as mybir
from concourse import tile
from concourse.alu_op_type import AluOpType
from concourse.bass_utils import run_bass_kernel_spmd

BF16 = ml_dtypes.bfloat16
F32 = mybir.dt.float32
BF = mybir.dt.bfloat16
AF = mybir.ActivationFunctionType
AX = mybir.AxisListType

N, T, D, H, L, R = 64, 128, 512, 1024, 16, 8
HS, GS = H // R, 4 * H // R  # 128, 512
GH = GS // 2  # 256: gate-half width for the packed B layout
SCALE = 1.0 / np.sqrt(H)


def _split_waits(nc, cap=1):
    """Walrus here rejects >cap sync waits per instruction; hoist extras
    onto preceding same-engine NOPs."""
    ctr = 0
    for fn in nc.m.functions:
        for bb in fn.blocks:
            out, changed = [], False
            for ins in bb.instructions:
                si = ins.sync_info
                if si is not None and si.on_wait and len(si.on_wait) > cap:
                    waits = list(si.on_wait)
                    extra, keep = waits[:-cap], waits[-cap:]
                    for i in range(0, len(extra), cap):
                        out.append(bass_rust.InstNoOp(
                            name=f"zz_waitsplit_{ctr}", engine=ins.engine,
                            sync_info=bass_rust.SyncInfo(
                                on_wait=extra[i:i + cap], on_update=[])))
                        ctr += 1
                    ins.sync_info = bass_rust.SyncInfo(
                        on_wait=keep, on_update=list(si.on_update or []))
                    changed = True
                out.append(ins)
            if changed:
                bb.instructions = out
    return ctr


def _prep_inputs(x, A, Wx, Wh, Wattn, b):
    x = np.asarray(x, np.float32)
    A_flat = np.asarray(A, np.float32).reshape(N, H, L)
    Wx = np.asarray(Wx, np.float32)
    Wh = np.asarray(Wh, np.float32)
    Wattn = np.asarray(Wattn, np.float32)
    b = np.asarray(b, np.float32)

    h0 = A_flat.mean(axis=2).astype(np.float32)
    scores0 = (np.einsum('nh,nhl->nl', h0, A_flat) * SCALE).astype(np.float32)
    xT = np.ascontiguousarray(
        x.transpose(1, 2, 0).reshape(T, 4, 128, N)).astype(BF16)
    h0T = np.ascontiguousarray(
        h0.T.reshape(8, 128, N).transpose(1, 0, 2)).astype(BF16)
    # A arranged for the on-device B precompute: lhsT chunks
    # [hh (partition), k (h-chunk), (m, nn, l)] with n = 8m + nn.
    abT = np.ascontiguousarray(
        A_flat.reshape(8, 8, 8, 128, L)            # [m, nn, k, hh, l]
        .transpose(2, 3, 0, 1, 4)                  # [k, hh, m, nn, l]
        .reshape(8, 128, 8 * 8 * L)).astype(BF16)  # [k, hh, 1024]
    idnstk = np.vstack([np.eye(N, dtype=np.float32)] * 2)  # [128, 64]
    s02 = np.tile(scores0, (2, 1))  # [128, 16]

    in_maps = []
    for j in range(R):
        cols = np.array([g * H + j * HS + i for g in range(4) for i in range(HS)])
        hsl = slice(j * HS, (j + 1) * HS)
        in_maps.append({
            "xT": xT,
            "whj": np.ascontiguousarray(
                Wh[:, cols].reshape(8, 128, GS)).astype(BF16),
            "wxj": np.ascontiguousarray(
                Wx[:, cols].reshape(4, 128, GS)).astype(BF16),
            "wanj": np.ascontiguousarray(
                Wattn[:, cols].reshape(8, 128, GS)).astype(BF16),
            # bias folded into B (exact: softmax weights sum to 1); packed
            # rows: partition p = n + 64*gh holds gate cols [gh*GH, gh*GH+GH)
            "b2rep": np.ascontiguousarray(
                np.tile(b[cols].reshape(2, GH), (N, 1)).reshape(N, 2, GH)
                .transpose(1, 0, 2).reshape(128, GH)).astype(np.float32),
            "abT": abT,
            "alh": np.ascontiguousarray(
                A_flat[:, hsl, :].transpose(0, 2, 1) * SCALE).astype(BF16),
            "idnstk": idnstk.astype(BF16),
            "h0T": h0T,
            "c0": np.ascontiguousarray(h0[:, hsl]),
            "s02": s02,
        })
    return in_maps


def _build():
    nc = bass.Bass("TRN2", target_bir_lowering=False, debug=False, num_devices=R)
    rg = [list(range(R))]

    xT_d = nc.dram_tensor("xT", [T, 4, 128, N], BF, kind="ExternalInput")
    whj_d = nc.dram_tensor("whj", [8, 128, GS], BF, kind="ExternalInput")
    wxj_d = nc.dram_tensor("wxj", [4, 128, GS], BF, kind="ExternalInput")
    wanj_d = nc.dram_tensor("wanj", [8, 128, GS], BF, kind="ExternalInput")
    b2rep_d = nc.dram_tensor("b2rep", [128, GH], F32, kind="ExternalInput")
    abT_d = nc.dram_tensor("abT", [8, 128, 8 * 8 * L], BF, kind="ExternalInput")
    alh_d = nc.dram_tensor("alh", [N, L, HS], BF, kind="ExternalInput")
    idn_d = nc.dram_tensor("idnstk", [128, N], BF, kind="ExternalInput")
    h0T_d = nc.dram_tensor("h0T", [128, 8, N], BF, kind="ExternalInput")
    c0_d = nc.dram_tensor("c0", [N, HS], F32, kind="ExternalInput")
    s02_d = nc.dram_tensor("s02", [128, L], F32, kind="ExternalInput")
    out_d = nc.dram_tensor("out", [N, T, HS], BF, kind="ExternalOutput")

    with tile.TileContext(nc) as tc:
        with tc.tile_pool(name="const", bufs=1) as cp, \
             tc.tile_pool(name="state", bufs=1) as st, \
             tc.tile_pool(name="dram", bufs=2, space="DRAM") as dp:

            whj = cp.tile([128, 8, GS], BF, name="whj")
            wxj = cp.tile([128, 4, GS], BF, name="wxj")
            alh = cp.tile([N, L, HS], BF, name="alh")
            idnstk = cp.tile([128, N], BF, name="idnstk")
            B2 = cp.tile([128, L, GH], BF, name="B2")
            nc.sync.dma_start(out=whj[:, :, :], in_=whj_d.rearrange("k p g -> p k g"))
            nc.sync.dma_start(out=wxj[:, :, :], in_=wxj_d.rearrange("k p g -> p k g"))
            nc.sync.dma_start(out=alh[:, :, :], in_=alh_d[:, :, :])
            nc.sync.dma_start(out=idnstk[:, :], in_=idn_d[:, :])

            c = st.tile([N, HS], F32, name="c")
            nc.sync.dma_start(out=c[:, :], in_=c0_d[:, :])
            houtT = st.tile([N, T, HS], BF, name="houtT")
            # block-diagonal staging for the attn-gate PSUM injection:
            # rows 0:64 carry gate cols 0:GH, rows 64:128 carry GH:GS;
            # off-diagonal blocks stay zero forever.
            ga_bd = st.tile([128, GS], BF, name="ga_bd")
            nc.vector.memset(ga_bd[:, :], 0.0)

            # ---- B precompute: B2[(gh n), l, gp] = B[n, l, gh*GH+gp] ----
            with tc.tile_pool(name="btmp", bufs=1) as bt, \
                 tc.tile_pool(name="ps_b", bufs=2, space="PSUM") as psb:
                b2_d = dp.tile([8, 128, GS], BF, name="b2stage")
                abT = bt.tile([128, 8, 8 * 8 * L], BF, name="abT")
                wan = bt.tile([128, 8, GS], BF, name="wan")
                b2rep = bt.tile([128, GH], F32, name="b2rep")
                nc.sync.dma_start(out=abT[:, :, :],
                                  in_=abT_d.rearrange("k p q -> p k q"))
                nc.sync.dma_start(out=wan[:, :, :],
                                  in_=wanj_d.rearrange("k p g -> p k g"))
                nc.sync.dma_start(out=b2rep[:, :], in_=b2rep_d[:, :])
                for m in range(8):
                    psB = psb.tile([128, GS], F32, name="psB", tag="psB")
                    for k in range(8):
                        nc.tensor.matmul(psB[:, :],
                                         abT[:, k, m * 128:(m + 1) * 128],
                                         wan[:, k, :],
                                         start=(k == 0), stop=(k == 7))
                    chunk = bt.tile([128, GS], BF, name=f"chunk{m}")
                    nc.vector.tensor_copy(out=chunk[:, :], in_=psB[:, :])
                    nc.sync.dma_start(out=b2_d[m], in_=chunk[:, :])
                b2view = b2_d.rearrange("m (nn l) (gh gp) -> gh (m nn) l gp",
                                        nn=8, l=L, gh=2, gp=GH)
                nc.sync.dma_start(out=B2[0:N, :, :], in_=b2view[0])
                nc.sync.dma_start(out=B2[N:128, :, :], in_=b2view[1])
                # fold the gate bias into B (softmax weights sum to 1)
                nc.vector.tensor_tensor(
                    out=B2[:, :, :], in0=B2[:, :, :],
                    in1=b2rep[:, None, :].broadcast_to((128, L, GH)),
                    op=AluOpType.add)

            with tc.tile_pool(name="wk", bufs=2) as wk, \
                 tc.tile_pool(name="ps_a", bufs=2, space="PSUM") as ps_a, \
                 tc.tile_pool(name="ps_t", bufs=2, space="PSUM") as ps_t:

                hT_full = wk.tile([128, 8, N], BF, name="hT0", tag="hT_full")
                nc.sync.dma_start(out=hT_full[:, :, :], in_=h0T_d[:, :, :])
                scores2 = wk.tile([128, L], F32, name="scores0", tag="scores2")
                nc.sync.dma_start(out=scores2[:, :], in_=s02_d[:, :])

                for t in range(T):
                    # ---- gate preacts accumulate in PSUM: x, bias, h ----
                    xtile = wk.tile([128, 4, N], BF, name="xtile", tag="xtile")
                    nc.sync.dma_start(out=xtile[:, :, :],
                                      in_=xT_d[t].rearrange("k p n -> p k n"))
                    pa = ps_a.tile([N, GS], F32, name="pa", tag="pa")
                    for kt in range(4):
                        nc.tensor.matmul(pa[:, :], xtile[:, kt, :], wxj[:, kt, :],
                                         start=(kt == 0), stop=False)
                    for kt in range(8):
                        nc.tensor.matmul(pa[:, :], hT_full[:, kt, :],
                                         whj[:, kt, :], start=False, stop=False)

                    # ---- softmax on duplicated [128, L] scores (real Exp;
                    # Exp and Tanh share the exp_and_others table) ----
                    e2 = wk.tile([128, L], BF, name="e2", tag="e2")
                    se2 = wk.tile([128, 1], F32, name="se2", tag="se2")
                    nc.scalar.activation(e2[:, :], scores2[:, :], AF.Exp,
                                         accum_out=se2[:, :])
                    rse2 = wk.tile([128, 1], F32, name="rse2", tag="rse2")
                    nc.vector.reciprocal(out=rse2[:, :], in_=se2[:, :])

                    # ---- attention-gate term via B: sum_l w_l * B_l ----
                    ga_m = wk.tile([128, L, GH], BF, name="ga_m", tag="ga_m")
                    nc.vector.scalar_tensor_tensor(
                        out=ga_m[:, :, :], in0=B2[:, :, :],
                        scalar=rse2[:, 0:1],
                        in1=e2[:, :, None].broadcast_to((128, L, GH)),
                        op0=AluOpType.mult, op1=AluOpType.mult)
                    ga2 = wk.tile([128, GH], BF, name="ga2", tag="ga2")
                    with nc.allow_low_precision(reason="16-term attn mix, bf16 ok"):
                        nc.vector.reduce_sum(
                            out=ga2[:, :],
                            in_=ga_m.rearrange("p l g -> p g l"),
                            axis=AX.X)
                    # partition-aligned copies into the block-diagonal tile
                    nc.scalar.copy(out=ga_bd[0:N, 0:GH], in_=ga2[0:N, :])
                    nc.scalar.copy(out=ga_bd[N:128, GH:GS], in_=ga2[N:128, :])
                    # inject into the PSUM gate accumulation (identity matmul)
                    nc.tensor.matmul(pa[:, :], idnstk[:, :], ga_bd[:, :],
                                     start=False, stop=True)

                    # ---- gates: i,f,o = sigmoid, g = tanh (all via Tanh) ----
                    th3 = wk.tile([N, 3 * HS], F32, name="th3", tag="th3")
                    nc.scalar.activation(th3[:, :], pa[:, 0:3 * HS], AF.Tanh,
                                         scale=0.5)
                    sig = wk.tile([N, 3 * HS], F32, name="sig", tag="sig")
                    nc.vector.tensor_scalar(out=sig[:, :], in0=th3[:, :],
                                            scalar1=1.0, scalar2=0.5,
                                            op0=AluOpType.add, op1=AluOpType.mult)
                    gt = wk.tile([N, HS], F32, name="gt", tag="gt")
                    nc.scalar.activation(gt[:, :], pa[:, 3 * HS:4 * HS], AF.Tanh)
                    t1 = wk.tile([N, HS], F32, name="t1", tag="t1")
                    nc.vector.tensor_mul(out=t1[:, :], in0=sig[:, 0:HS],
                                         in1=gt[:, :])
                    nc.vector.tensor_mul(out=c[:, :], in0=sig[:, HS:2 * HS],
                                         in1=c[:, :])
                    nc.vector.tensor_add(out=c[:, :], in0=c[:, :], in1=t1[:, :])
                    tanc = wk.tile([N, HS], F32, name="tanc", tag="tanc")
                    nc.scalar.activation(tanc[:, :], c[:, :], AF.Tanh)
                    nc.vector.tensor_mul(out=houtT[:, t, :],
                                         in0=sig[:, 2 * HS:3 * HS],
                                         in1=tanc[:, :])
                    if t == T - 1:
                        break

                    # ---- next-step comms: hT slice + score partials ----
                    pt = ps_t.tile([128, N], BF, name="pt", tag="pt")
                    nc.tensor.transpose(pt[:, :], houtT[:, t, :], idnstk[0:N, :])
                    hT_bf = wk.tile([128, N], BF, name="hT_bf", tag="hT_bf")
                    nc.vector.tensor_copy(out=hT_bf[:, :], in_=pt[:, :])
                    atm = wk.tile([N, L, HS], BF, name="atm", tag="atm")
                    nc.vector.tensor_tensor(
                        out=atm[:, :, :], in0=alh[:, :, :],
                        in1=houtT[:, t, None, :].broadcast_to((N, L, HS)),
                        op=AluOpType.mult)
                    spart = wk.tile([N, L], BF, name="spart", tag="spart")
                    with nc.allow_low_precision(reason="score partials, bf16 ok"):
                        nc.vector.reduce_sum(out=spart[:, :], in_=atm[:, :, :],
                                             axis=AX.X)

                    sendA = dp.tile([8192 + N * L], BF, name="sendA", tag="sendA")
                    recvA = dp.tile([R, 8192 + N * L], BF, name="recvA",
                                    tag="recvA", addr_space="Shared")
                    nc.sync.dma_start(
                        out=sendA[0:8192].rearrange("(p n) -> p n", p=128),
                        in_=hT_bf[:, :])
                    nc.sync.dma_start(
                        out=sendA[8192:8192 + N * L].rearrange("(n l) -> n l", n=N),
                        in_=spart[:, :])
                    nc.gpsimd.collective_compute(
                        "AllGather", AluOpType.bypass, replica_groups=rg,
                        ins=[sendA[:].opt()], outs=[recvA[:, :].opt()])
                    hT_full = wk.tile([128, 8, N], BF, name="hT_full",
                                      tag="hT_full")
                    nc.scalar.dma_start(
                        out=hT_full[:, :, :],
                        in_=recvA[:, 0:8192].rearrange("r (p n) -> p r n", p=128))
                    sp2 = wk.tile([128, 8, L], BF, name="sp2", tag="sp2")
                    nc.sync.dma_start(
                        out=sp2[0:N, :, :],
                        in_=recvA[:, 8192:8192 + N * L].rearrange(
                            "r (n l) -> n r l", n=N))
                    nc.sync.dma_start(
                        out=sp2[N:128, :, :],
                        in_=recvA[:, 8192:8192 + N * L].rearrange(
                            "r (n l) -> n r l", n=N))
                    scores2 = wk.tile([128, L], F32, name="scores2",
                                      tag="scores2")
                    nc.vector.reduce_sum(out=scores2[:, :],
                                         in_=sp2.rearrange("p r l -> p l r"),
                                         axis=AX.X)

                nc.sync.dma_start(out=out_d[:, :, :], in_=houtT[:, :, :])

    _split_waits(nc, cap=1)
    return nc


_NC_CACHE = None


def kernel(**inputs) -> np.ndarray:
    global _NC_CACHE
    in_maps = _prep_inputs(**inputs)
    if _NC_CACHE is None:
        _NC_CACHE = _build()
    res = run_bass_kernel_spmd(_NC_CACHE, in_maps, core_ids=list(range(R)))
    out = np.zeros((N, T, H), dtype=np.float32)
    for j, r in enumerate(res.results):
        out[:, :, j * HS:(j + 1) * HS] = \
            np.asarray(r["out"]).astype(np.float32).reshape(N, T, HS)
    return out


# revision 27
# speedup vs baseline: 1.3930x; 1.0704x over previous
"""Trainium2 Bass kernel for nn_CaptioningRNN (attention-LSTM over T=128 steps).

Sharding: tensor-parallel over the 4H gate dimension across 8 NeuronCores.
Core j owns H-slice j (128 h-rows) of each of the 4 gates, so the per-step
LSTM cell state (c, h) for that slice lives entirely on core j.

Key structure (v2 — ONE collective per step):
  - The attention contribution to the gates is reparametrized through
    B[n, l, g] = sum_h A[n, h, l] * Wattn[h, g]  (precomputed on-device,
    per-core gate slice, packed [128, 256, 16] across two gate halves), so
    per step  attn @ Wattn == sum_l softmax(scores)[n, l] * B[n, l, :] —
    a small DVE contraction over L=16 on all 128 partitions. This removes
    the per-step attn^T AllGather entirely.
  - Per step each core broadcasts (hT slice | score partials) in ONE bf16
    AllGather; scores are summed from partials, softmax is computed
    duplicated on 128 partitions, and the attention-gate term is injected
    into the PSUM gate accumulation via two tiny identity matmuls.
  - The x_t @ Wx gate ktiles + a K=1 bias ktile ride in the same PSUM
    accumulation during the AllGather window.
  - All activations go through Tanh only (sigmoid/exp via tanh identities)
    so ScalarE keeps one activation table loaded.
  - h is accumulated in SBUF (bf16) and written out with a single DMA at
    the end. Host side does layout-only prep and output assembly.
"""
import numpy as np
import ml_dtypes

import bass_rust
import concourse.bass as bass
import concourse.mybir as mybir
from concourse import tile
from concourse.alu_op_type import AluOpType
from concourse.bass_utils import run_bass_kernel_spmd

BF16 = ml_dtypes.bfloat16
F32 = mybir.dt.float32
BF = mybir.dt.bfloat16
AF = mybir.ActivationFunctionType
AX = mybir.AxisListType

N, T, D, H, L, R = 64, 128, 512, 1024, 16, 8
HS, GS = H // R, 4 * H // R  # 128, 512
GH = GS // 2  # 256: gate-half width for the packed B layout
SCALE = 1.0 / np.sqrt(H)


def _split_waits(nc, cap=1):
    """Walrus here rejects >cap sync waits per instruction; hoist extras
    onto preceding same-engine NOPs."""
    ctr = 0
    for fn in nc.m.functions:
        for bb in fn.blocks:
            out, changed = [], False
            for ins in bb.instructions:
                si = ins.sync_info
                if si is not None and si.on_wait and len(si.on_wait) > cap:
                    waits = list(si.on_wait)
                    extra, keep = waits[:-cap], waits[-cap:]
                    for i in range(0, len(extra), cap):
                        out.append(bass_rust.InstNoOp(
                            name=f"zz_waitsplit_{ctr}", engine=ins.engine,
                            sync_info=bass_rust.SyncInfo(
                                on_wait=extra[i:i + cap], on_update=[])))
                        ctr += 1
                    ins.sync_info = bass_rust.SyncInfo(
                        on_wait=keep, on_update=list(si.on_update or []))
                    changed = True
                out.append(ins)
            if changed:
                bb.instructions = out
    return ctr


def _prep_inputs(x, A, Wx, Wh, Wattn, b):
    x = np.asarray(x, np.float32)
    A_flat = np.asarray(A, np.float32).reshape(N, H, L)
    Wx = np.asarray(Wx, np.float32)
    Wh = np.asarray(Wh, np.float32)
    Wattn = np.asarray(Wattn, np.float32)
    b = np.asarray(b, np.float32)

    h0 = A_flat.mean(axis=2).astype(np.float32)
    scores0 = (np.einsum('nh,nhl->nl', h0, A_flat) * SCALE).astype(np.float32)
    xT = np.ascontiguousarray(
        x.transpose(1, 2, 0).reshape(T, 4, 128, N)).astype(BF16)
    h0T = np.ascontiguousarray(
        h0.T.reshape(8, 128, N).transpose(1, 0, 2)).astype(BF16)
    # A arranged for the on-device B precompute: lhsT chunks
    # [hh (partition), k (h-chunk), (m, nn, l)] with n = 8m + nn.
    abT = np.ascontiguousarray(
        A_flat.reshape(8, 8, 8, 128, L)            # [m, nn, k, hh, l]
        .transpose(2, 3, 0, 1, 4)                  # [k, hh, m, nn, l]
        .reshape(8, 128, 8 * 8 * L)).astype(BF16)  # [k, hh, 1024]
    idnstk = np.vstack([np.eye(N, dtype=np.float32)] * 2)  # [128, 64]
    s02 = np.tile(scores0, (2, 1))  # [128, 16]

    in_maps = []
    for j in range(R):
        cols = np.array([g * H + j * HS + i for g in range(4) for i in range(HS)])
        hsl = slice(j * HS, (j + 1) * HS)
        in_maps.append({
            "xT": xT,
            "whj": np.ascontiguousarray(
                Wh[:, cols].reshape(8, 128, GS)).astype(BF16),
            "wxj": np.ascontiguousarray(
                Wx[:, cols].reshape(4, 128, GS)).astype(BF16),
            "wanj": np.ascontiguousarray(
                Wattn[:, cols].reshape(8, 128, GS)).astype(BF16),
            # bias folded into B (exact: softmax weights sum to 1); packed
            # rows: partition p = n + 64*gh holds gate cols [gh*GH, gh*GH+GH)
            "b2rep": np.ascontiguousarray(
                np.tile(b[cols].reshape(2, GH), (N, 1)).reshape(N, 2, GH)
                .transpose(1, 0, 2).reshape(128, GH)).astype(np.float32),
            "abT": abT,
            "alh": np.ascontiguousarray(
                A_flat[:, hsl, :].transpose(0, 2, 1) * SCALE).astype(BF16),
            "idnstk": idnstk.astype(BF16),
            "idn128": np.eye(128, dtype=BF16),
            "h0T": h0T,
            "c0": np.ascontiguousarray(h0[:, hsl]),
            "s02": s02,
        })
    return in_maps


def _build():
    nc = bass.Bass("TRN2", target_bir_lowering=False, debug=False, num_devices=R)
    rg = [list(range(R))]

    xT_d = nc.dram_tensor("xT", [T, 4, 128, N], BF, kind="ExternalInput")
    whj_d = nc.dram_tensor("whj", [8, 128, GS], BF, kind="ExternalInput")
    wxj_d = nc.dram_tensor("wxj", [4, 128, GS], BF, kind="ExternalInput")
    wanj_d = nc.dram_tensor("wanj", [8, 128, GS], BF, kind="ExternalInput")
    b2rep_d = nc.dram_tensor("b2rep", [128, GH], F32, kind="ExternalInput")
    abT_d = nc.dram_tensor("abT", [8, 128, 8 * 8 * L], BF, kind="ExternalInput")
    alh_d = nc.dram_tensor("alh", [N, L, HS], BF, kind="ExternalInput")
    idn_d = nc.dram_tensor("idnstk", [128, N], BF, kind="ExternalInput")
    idn128_d = nc.dram_tensor("idn128", [128, 128], BF, kind="ExternalInput")
    h0T_d = nc.dram_tensor("h0T", [128, 8, N], BF, kind="ExternalInput")
    c0_d = nc.dram_tensor("c0", [N, HS], F32, kind="ExternalInput")
    s02_d = nc.dram_tensor("s02", [128, L], F32, kind="ExternalInput")
    out_d = nc.dram_tensor("out", [N, T, HS], BF, kind="ExternalOutput")

    with tile.TileContext(nc) as tc:
        with tc.tile_pool(name="const", bufs=1) as cp, \
             tc.tile_pool(name="state", bufs=1) as st, \
             tc.tile_pool(name="dram", bufs=2, space="DRAM") as dp:

            whj = cp.tile([128, 8, GS], BF, name="whj")
            wxj = cp.tile([128, 4, GS], BF, name="wxj")
            alh = cp.tile([N, L, HS], BF, name="alh")
            idnstk = cp.tile([128, N], BF, name="idnstk")
            B2 = cp.tile([128, GH, L], BF, name="B2")
            nc.sync.dma_start(out=whj[:, :, :], in_=whj_d.rearrange("k p g -> p k g"))
            nc.sync.dma_start(out=wxj[:, :, :], in_=wxj_d.rearrange("k p g -> p k g"))
            nc.sync.dma_start(out=alh[:, :, :], in_=alh_d[:, :, :])
            nc.sync.dma_start(out=idnstk[:, :], in_=idn_d[:, :])

            c = st.tile([N, HS], F32, name="c")
            nc.sync.dma_start(out=c[:, :], in_=c0_d[:, :])
            houtT = st.tile([N, T, HS], BF, name="houtT")
            # block-diagonal staging for the attn-gate PSUM injection:
            # rows 0:64 carry gate cols 0:GH, rows 64:128 carry GH:GS;
            # off-diagonal blocks stay zero forever.
            ga_bd = st.tile([128, GS], BF, name="ga_bd")
            nc.vector.memset(ga_bd[:, :], 0.0)

            # ---- B precompute: B2[(gh n), gp, l] = B[n, l, gh*GH+gp] ----
            # PE-transpose each [128,128] sub-block of the chunks so the
            # spatial axis l lands innermost-contiguous in B2.
            with tc.tile_pool(name="btmp", bufs=1) as bt, \
                 tc.tile_pool(name="ps_b", bufs=2, space="PSUM") as psb:
                cs_d = dp.tile([4, 128, 8, 128], BF, name="b2stage")
                abT = bt.tile([128, 8, 8 * 8 * L], BF, name="abT")
                wan = bt.tile([128, 8, GS], BF, name="wan")
                idn128 = bt.tile([128, 128], BF, name="idn128")
                b2rep = bt.tile([128, GH], F32, name="b2rep")
                nc.sync.dma_start(out=abT[:, :, :],
                                  in_=abT_d.rearrange("k p q -> p k q"))
                nc.sync.dma_start(out=wan[:, :, :],
                                  in_=wanj_d.rearrange("k p g -> p k g"))
                nc.sync.dma_start(out=idn128[:, :], in_=idn128_d[:, :])
                nc.sync.dma_start(out=b2rep[:, :], in_=b2rep_d[:, :])
                for m in range(8):
                    psB = psb.tile([128, GS], F32, name="psB", tag="psB")
                    for k in range(8):
                        nc.tensor.matmul(psB[:, :],
                                         abT[:, k, m * 128:(m + 1) * 128],
                                         wan[:, k, :],
                                         start=(k == 0), stop=(k == 7))
                    chunk = bt.tile([128, GS], BF, name=f"chunk{m}")
                    nc.vector.tensor_copy(out=chunk[:, :], in_=psB[:, :])
                    for cb in range(4):
                        psT = psb.tile([128, 128], BF, name="psT", tag="psT")
                        nc.tensor.transpose(
                            psT[:, :], chunk[:, cb * 128:(cb + 1) * 128],
                            idn128[:, :])
                        ctp = bt.tile([128, 128], BF, name=f"ctp{m}_{cb}")
                        nc.vector.tensor_copy(out=ctp[:, :], in_=psT[:, :])
                        nc.sync.dma_start(out=cs_d[cb, :, m, :], in_=ctp[:, :])
                # cs_d[cb=(gh gpc), gg, m, q=(nn l)]; collapse (m nn), (gpc gg)
                csview = cs_d.rearrange(
                    "(gh gpc) gg m (nn l) -> gh (m nn) (gpc gg) l",
                    gh=2, gpc=2, nn=8, l=L)
                nc.sync.dma_start(out=B2[0:N, :, :], in_=csview[0])
                nc.sync.dma_start(out=B2[N:128, :, :], in_=csview[1])
                # fold the gate bias into B (softmax weights sum to 1)
                nc.vector.tensor_tensor(
                    out=B2[:, :, :], in0=B2[:, :, :],
                    in1=b2rep[:, :, None].broadcast_to((128, GH, L)),
                    op=AluOpType.add)

            with tc.tile_pool(name="wk", bufs=2) as wk, \
                 tc.tile_pool(name="ps_a", bufs=2, space="PSUM") as ps_a, \
                 tc.tile_pool(name="ps_t", bufs=2, space="PSUM") as ps_t:

                hT_full = wk.tile([128, 8, N], BF, name="hT0", tag="hT_full")
                nc.sync.dma_start(out=hT_full[:, :, :], in_=h0T_d[:, :, :])
                scores2 = wk.tile([128, L], F32, name="scores0", tag="scores2")
                nc.sync.dma_start(out=scores2[:, :], in_=s02_d[:, :])

                for t in range(T):
                    # ---- gate preacts accumulate in PSUM: x, bias, h ----
                    xtile = wk.tile([128, 4, N], BF, name="xtile", tag="xtile")
                    nc.sync.dma_start(out=xtile[:, :, :],
                                      in_=xT_d[t].rearrange("k p n -> p k n"))
                    pa = ps_a.tile([N, GS], F32, name="pa", tag="pa")
                    for kt in range(4):
                        nc.tensor.matmul(pa[:, :], xtile[:, kt, :], wxj[:, kt, :],
                                         start=(kt == 0), stop=False)
                    for kt in range(8):
                        nc.tensor.matmul(pa[:, :], hT_full[:, kt, :],
                                         whj[:, kt, :], start=False, stop=False)

                    # ---- softmax on duplicated [128, L] scores (real Exp;
                    # Exp and Tanh share the exp_and_others table) ----
                    e2 = wk.tile([128, L], BF, name="e2", tag="e2")
                    se2 = wk.tile([128, 1], F32, name="se2", tag="se2")
                    nc.scalar.activation(e2[:, :], scores2[:, :], AF.Exp,
                                         accum_out=se2[:, :])
                    rse2 = wk.tile([128, 1], F32, name="rse2", tag="rse2")
                    nc.vector.reciprocal(out=rse2[:, :], in_=se2[:, :])

                    # ---- attention-gate term via B: sum_l w_l * B_l ----
                    ga_m = wk.tile([128, GH, L], BF, name="ga_m", tag="ga_m")
                    nc.vector.scalar_tensor_tensor(
                        out=ga_m[:, :, :], in0=B2[:, :, :],
                        scalar=rse2[:, 0:1],
                        in1=e2[:, None, :].broadcast_to((128, GH, L)),
                        op0=AluOpType.mult, op1=AluOpType.mult)
                    ga2 = wk.tile([128, GH], BF, name="ga2", tag="ga2")
                    with nc.allow_low_precision(reason="16-term attn mix, bf16 ok"):
                        nc.vector.reduce_sum(out=ga2[:, :], in_=ga_m[:, :, :],
                                             axis=AX.X)
                    # partition-aligned copies into the block-diagonal tile
                    nc.scalar.copy(out=ga_bd[0:N, 0:GH], in_=ga2[0:N, :])
                    nc.scalar.copy(out=ga_bd[N:128, GH:GS], in_=ga2[N:128, :])
                    # inject into the PSUM gate accumulation (identity matmul)
                    nc.tensor.matmul(pa[:, :], idnstk[:, :], ga_bd[:, :],
                                     start=False, stop=True)

                    # ---- gates: i,f,o = sigmoid, g = tanh (all via Tanh) ----
                    th3 = wk.tile([N, 3 * HS], F32, name="th3", tag="th3")
                    nc.scalar.activation(th3[:, :], pa[:, 0:3 * HS], AF.Tanh,
                                         scale=0.5)
                    sig = wk.tile([N, 3 * HS], F32, name="sig", tag="sig")
                    nc.vector.tensor_scalar(out=sig[:, :], in0=th3[:, :],
                                            scalar1=1.0, scalar2=0.5,
                                            op0=AluOpType.add, op1=AluOpType.mult)
                    gt = wk.tile([N, HS], F32, name="gt", tag="gt")
                    nc.scalar.activation(gt[:, :], pa[:, 3 * HS:4 * HS], AF.Tanh)
                    t1 = wk.tile([N, HS], F32, name="t1", tag="t1")
                    nc.vector.tensor_mul(out=t1[:, :], in0=sig[:, 0:HS],
                                         in1=gt[:, :])
                    nc.vector.tensor_mul(out=c[:, :], in0=sig[:, HS:2 * HS],
                                         in1=c[:, :])
                    nc.vector.tensor_add(out=c[:, :], in0=c[:, :], in1=t1[:, :])
                    tanc = wk.tile([N, HS], F32, name="tanc", tag="tanc")
                    nc.scalar.activation(tanc[:, :], c[:, :], AF.Tanh)
                    nc.vector.tensor_mul(out=houtT[:, t, :],
                                         in0=sig[:, 2 * HS:3 * HS],
                                         in1=tanc[:, :])
                    if t == T - 1:
                        break

                    # ---- next-step comms: hT slice + score partials ----
                    pt = ps_t.tile([128, N], BF, name="pt", tag="pt")
                    nc.tensor.transpose(pt[:, :], houtT[:, t, :], idnstk[0:N, :])
                    hT_bf = wk.tile([128, N], BF, name="hT_bf", tag="hT_bf")
                    nc.vector.tensor_copy(out=hT_bf[:, :], in_=pt[:, :])
                    atm = wk.tile([N, L, HS], BF, name="atm", tag="atm")
                    nc.vector.tensor_tensor(
                        out=atm[:, :, :], in0=alh[:, :, :],
                        in1=houtT[:, t, None, :].broadcast_to((N, L, HS)),
                        op=AluOpType.mult)
                    spart = wk.tile([N, L], BF, name="spart", tag="spart")
                    with nc.allow_low_precision(reason="score partials, bf16 ok"):
                        nc.vector.reduce_sum(out=spart[:, :], in_=atm[:, :, :],
                                             axis=AX.X)

                    sendA = dp.tile([8192 + N * L], BF, name="sendA", tag="sendA")
                    recvA = dp.tile([R, 8192 + N * L], BF, name="recvA",
                                    tag="recvA", addr_space="Shared")
                    nc.sync.dma_start(
                        out=sendA[0:8192].rearrange("(p n) -> p n", p=128),
                        in_=hT_bf[:, :])
                    nc.sync.dma_start(
                        out=sendA[8192:8192 + N * L].rearrange("(n l) -> n l", n=N),
                        in_=spart[:, :])
                    nc.gpsimd.collective_compute(
                        "AllGather", AluOpType.bypass, replica_groups=rg,
                        ins=[sendA[:].opt()], outs=[recvA[:, :].opt()])
                    hT_full = wk.tile([128, 8, N], BF, name="hT_full",
                                      tag="hT_full")
                    nc.scalar.dma_start(
                        out=hT_full[:, :, :],
                        in_=recvA[:, 0:8192].rearrange("r (p n) -> p r n", p=128))
                    sp2 = wk.tile([128, 8, L], BF, name="sp2", tag="sp2")
                    nc.sync.dma_start(
                        out=sp2[0:N, :, :],
                        in_=recvA[:, 8192:8192 + N * L].rearrange(
                            "r (n l) -> n r l", n=N))
                    nc.sync.dma_start(
                        out=sp2[N:128, :, :],
                        in_=recvA[:, 8192:8192 + N * L].rearrange(
                            "r (n l) -> n r l", n=N))
                    scores2 = wk.tile([128, L], F32, name="scores2",
                                      tag="scores2")
                    nc.vector.reduce_sum(out=scores2[:, :],
                                         in_=sp2.rearrange("p r l -> p l r"),
                                         axis=AX.X)

                nc.sync.dma_start(out=out_d[:, :, :], in_=houtT[:, :, :])

    _split_waits(nc, cap=1)
    return nc


_NC_CACHE = None


def kernel(**inputs) -> np.ndarray:
    global _NC_CACHE
    in_maps = _prep_inputs(**inputs)
    if _NC_CACHE is None:
        _NC_CACHE = _build()
    res = run_bass_kernel_spmd(_NC_CACHE, in_maps, core_ids=list(range(R)))
    out = np.zeros((N, T, H), dtype=np.float32)
    for j, r in enumerate(res.results):
        out[:, :, j * HS:(j + 1) * HS] = \
            np.asarray(r["out"]).astype(np.float32).reshape(N, T, HS)
    return out
